# revision 1
# baseline (speedup 1.0000x reference)
"""Causal attention head (RoPE) kernel for 8 Trainium2 NeuronCores.

Sharding: 8 cores = 2 batches x 4 head-groups (4 heads each), no
cross-device comms. Per core the device works in feature-major layout:

  - host passes xT = x[b].T (bf16), weights pre-transposed; Wq/Wk rows are
    permuted per head so RoPE even components land in partitions [0:32) and
    odd components in [32:64) of each head's 64-row block.
  - Q^T/K^T/V^T projected with wide 512-row matmuls over 8 c-tiles; RoPE
    applied as new = X*cos - swap(X*sin') (cos is swap-invariant and
    swap(sin') = -sin', so the partition swap runs on the SBUF sin-product
    via 4 block DMAs). V^T is turned into natural-layout V via PE transposes.
  - scores are built transposed (S^T[k,q] = K.Q^T) so softmax'd P^T feeds the
    PV matmul directly (no transposes); the two heads of a pair occupy
    partitions 0-63/64-127 and their K=64 score matmuls run on the two 64x128
    row tiles of the PE array. V carries a ones-column per head, so row 64 of
    the PV output is the softmax denominator for free. exp() runs on ACT with
    the 1/32 scale folded in; no max-subtraction (scores are ~N(0, 0.1)).
  - output O^T is normalized via reciprocal_approx + a K=1 fp16 broadcast
    matmul and written back transposed; the host re-transposes on gather.
"""

import os
import sys
from contextlib import ExitStack

import numpy as np

for _p in ("/opt/trn_rl_repo", "/root/.axon_site/_ro/trn_rl_repo"):
    if os.path.isdir(_p) and _p not in sys.path:
        sys.path.append(_p)

import ml_dtypes

import concourse.bass as bass
import concourse.mybir as mybir
import concourse.tile as tile
from concourse import bacc
from concourse.bass_utils import run_bass_kernel_spmd

P = 128
T = 2048
CIN = 1024
NHC = 4          # heads per core
HS = 64
DOUT = NHC * HS  # 256
NCT = CIN // P   # 8 contraction tiles
SCALE = 1.0 / 32.0  # 1024 ** -0.5

F32 = mybir.dt.float32
BF16 = mybir.dt.bfloat16


def _build_nc():
    nc = bacc.Bacc("TRN2")

    xT = nc.dram_tensor("xT", [CIN, T], BF16, kind="ExternalInput").ap()
    wqT = nc.dram_tensor("wqT", [CIN, DOUT], BF16, kind="ExternalInput").ap()
    wkT = nc.dram_tensor("wkT", [CIN, DOUT], BF16, kind="ExternalInput").ap()
    wvT = nc.dram_tensor("wvT", [CIN, DOUT], BF16, kind="ExternalInput").ap()
    cos4 = nc.dram_tensor("cos4", [P, T], F32, kind="ExternalInput").ap()
    sin4 = nc.dram_tensor("sin4", [P, T], F32, kind="ExternalInput").ap()
    utri = nc.dram_tensor("utri", [P, P], BF16, kind="ExternalInput").ap()
    ident = nc.dram_tensor("ident", [P, P], BF16, kind="ExternalInput").ap()
    outT = nc.dram_tensor("outT", [DOUT, T], F32, kind="ExternalOutput").ap()

    with tile.TileContext(nc) as tc, ExitStack() as ctx:
        const_pool = ctx.enter_context(tc.tile_pool(name="const", bufs=1))
        wpool = ctx.enter_context(tc.tile_pool(name="w", bufs=1))
        qkpool = ctx.enter_context(tc.tile_pool(name="qk", bufs=1))
        vpool = ctx.enter_context(tc.tile_pool(name="vaug", bufs=1))
        phase1 = ExitStack()
        xpool = phase1.enter_context(tc.tile_pool(name="x", bufs=1))
        tmppool = phase1.enter_context(tc.tile_pool(name="tmp", bufs=3))

        # ---- inputs to SBUF (x + weights first: the projection needs them
        # immediately; rope/mask constants are not used until ~40us in)
        w_tiles = {}
        for name, wsrc in (("v", wvT), ("q", wqT), ("k", wkT)):
            w_s = wpool.tile([P, NCT * DOUT], BF16, tag=f"w{name}", name=f"w{name}")
            nc.sync.dma_start(
                w_s.rearrange("p (n d) -> p n d", n=NCT), wsrc.rearrange("(n p) d -> p n d", p=P)
            )
            w_tiles[name] = w_s
        xs = xpool.tile([P, NCT * T], BF16, tag="xs")
        xT_r = xT.rearrange("(n p) t -> p n t", p=P)  # [128, 8, 2048]
        H2 = T // 2
        _dma_engs = (nc.sync, nc.gpsimd, nc.scalar)
        for c in range(NCT):
            for half in range(2):
                eng = _dma_engs[(2 * c + half) % 3]
                eng.dma_start(
                    xs[:, c * T + half * H2: c * T + (half + 1) * H2],
                    xT_r[:, c, half * H2:(half + 1) * H2],
                )
        cos_s = const_pool.tile([P, T], F32, tag="cos")
        nc.sync.dma_start(cos_s[:], cos4)
        sin_s = const_pool.tile([P, T], F32, tag="sin")
        nc.sync.dma_start(sin_s[:], sin4)
        utri_s = const_pool.tile([P, P], BF16, tag="utri")
        nc.sync.dma_start(utri_s[:], utri)
        ident_s = const_pool.tile([P, P], BF16, tag="ident")
        nc.sync.dma_start(ident_s[:], ident)
        ones64 = const_pool.tile([1, HS], mybir.dt.float16, tag="ones64")
        nc.vector.memset(ones64[:], 1.0)

        # ---- phase 1: all three projections as wide 512-row matmuls.
        # V is projected feature-major (V^T) like Q/K to avoid 128 narrow
        # weight-reloading matmuls, then turned into natural-layout V_aug
        # via 32 PE transposes.
        qt = [qkpool.tile([P, T], BF16, tag=f"qt{m}", name=f"qt{m}") for m in range(2)]
        kt = [qkpool.tile([P, T], BF16, tag=f"kt{m}", name=f"kt{m}") for m in range(2)]
        vts = [
            tmppool.tile([P, T], BF16, tag=f"vt{m}", name=f"vt{m}") for m in range(2)
        ]

        with tc.tile_pool(name="pp_proj", bufs=2, space="PSUM") as pp_proj:
            _order = [("v", vts, 0), ("v", vts, 1),
                      ("q", qt, 0), ("k", kt, 0), ("q", qt, 1), ("k", kt, 1)]
            for wname, dst, m in _order:
                w_s = w_tiles[wname]
                if True:
                    ps = pp_proj.tile([P, T], F32, tag="proj")
                    for c in range(NCT):
                        for nch in range(4):
                            sl = slice(nch * 512, (nch + 1) * 512)
                            nc.tensor.matmul(
                                ps[:, sl],
                                lhsT=w_s[:, c * DOUT + m * P: c * DOUT + (m + 1) * P],
                                rhs=xs[:, c * T + nch * 512: c * T + (nch + 1) * 512],
                                start=(c == 0),
                                stop=(c == NCT - 1),
                            )
                    if wname == "v":
                        for nch in range(2):
                            sl = slice(nch * 1024, (nch + 1) * 1024)
                            nc.vector.tensor_copy(dst[m][:, sl], ps[:, sl])
                        continue
                    # RoPE: new = ps*cos - swap(ps*sin'), full-tile granularity
                    # (cos4 is swap-invariant and swap(sin4n) = -sin4n, so the
                    #  partition swap runs on the SBUF sin-product via 4 DMAs)
                    a = tmppool.tile([P, T], F32, tag="ropeA")
                    ap = tmppool.tile([P, T], F32, tag="ropeAp")
                    for nch in range(4):
                        sl = slice(nch * 512, (nch + 1) * 512)
                        nc.vector.tensor_mul(a[:, sl], ps[:, sl], cos_s[:, sl])
                        nc.vector.tensor_mul(ap[:, sl], ps[:, sl], sin_s[:, sl])
                    sw = tmppool.tile([P, T], F32, tag="ropeS")
                    for blk in range(4):
                        s0 = (blk ^ 1) * 32
                        nc.sync.dma_start(
                            sw[blk * 32:(blk + 1) * 32, :], ap[s0:s0 + 32, :]
                        )
                    nc.gpsimd.tensor_sub(dst[m][:], a[:], sw[:])

        # V^T -> natural-layout V_aug tiles (ones column appended per head)
        va = []
        with tc.tile_pool(name="pp_tr", bufs=4, space="PSUM") as pp_tr:
            for t in range(T // P):
                vt = vpool.tile([P, NHC * (HS + 1)], BF16, tag=f"vaug{t}")
                vt_r = vt.rearrange("p (h e) -> p h e", e=HS + 1)
                nc.gpsimd.memset(vt_r[:, :, HS:HS + 1], 1.0)
                for m in range(2):
                    tr = pp_tr.tile([P, P], BF16, tag="tr")
                    nc.tensor.transpose(
                        tr[:], vts[m][:, t * P:(t + 1) * P], ident_s[:]
                    )
                    nc.vector.tensor_copy(
                        vt_r[:, 2 * m:2 * m + 2, 0:HS],
                        tr.rearrange("p (h d) -> p h d", d=HS),
                    )
                va.append(vt)

        phase1.close()  # release xs/tmp zones; PT tiles below reuse them

        # ---- phase 2: attention, two heads interleaved so PE keeps working
        # while ACT runs the other head's exp
        ptpool = ctx.enter_context(tc.tile_pool(name="pt", bufs=1))
        otpool = ctx.enter_context(tc.tile_pool(name="ot", bufs=3))
        rspool = ctx.enter_context(tc.tile_pool(name="rs", bufs=3))
        pp_s = ctx.enter_context(tc.tile_pool(name="pp_s", bufs=3, space="PSUM"))
        pp_o = ctx.enter_context(tc.tile_pool(name="pp_o", bufs=1, space="PSUM"))
        pp_b = ctx.enter_context(tc.tile_pool(name="pp_b", bufs=1, space="PSUM"))

        for pair in ((0, 1), (2, 3)):
            qt_t, kt_t = qt[pair[0] // 2], kt[pair[0] // 2]
            pts = {h: [] for h in pair}
            for j in range(T // P):
                w_j = T - j * P
                ptj_pair = []
                for hi, h in enumerate(pair):
                    ptj = ptpool.tile(
                        [P, w_j], BF16, tag=f"pt{hi}_{j}", name=f"pt{hi}_{j}",
                        bufs=2 if j < 6 else None,
                    )
                    pts[h].append(ptj)
                    ptj_pair.append(ptj)
                for seg0 in range(0, w_j, 1024):
                    seg = min(1024, w_j - seg0)
                    # both heads' score matmuls run concurrently on the two
                    # 64x128 row tiles of the PE array (K=64 each)
                    ps_pair = [
                        pp_s.tile([P, 1024], F32, tag="ps", name=f"ps{hi}")
                        for hi in range(2)
                    ]
                    for s5 in range(0, seg, 512):
                        n = min(512, seg - s5)
                        q0 = j * P + seg0 + s5
                        for hi in range(2):
                            r0 = hi * HS
                            nc.tensor.matmul(
                                ps_pair[hi][:, s5:s5 + n],
                                lhsT=kt_t[r0:r0 + HS, j * P:(j + 1) * P],
                                rhs=qt_t[r0:r0 + HS, q0:q0 + n],
                                start=True,
                                stop=True,
                                tile_position=(hi * HS, 0),
                            )
                    for hi in range(2):
                        nc.scalar.activation(
                            ptj_pair[hi][:, seg0:seg0 + seg],
                            ps_pair[hi][:, 0:seg],
                            mybir.ActivationFunctionType.Exp,
                            scale=SCALE,
                        )
                # causal mask on the diagonal block (col 0 = q-offset j*128)
                for hi in range(2):
                    nc.vector.tensor_mul(
                        ptj_pair[hi][:, 0:P], ptj_pair[hi][:, 0:P], utri_s[:]
                    )

                if j % 4 == 3:
                    qc = j // 4
                    q0 = qc * 512
                    jmax = j
                    for h in pair:
                        po = pp_o.tile([HS + 1, 512], F32, tag="po")
                        # full-width k-tiles first (jj*128 <= q0), partials after
                        order = [jj for jj in range(jmax + 1) if jj * P <= q0]
                        order += [jj for jj in range(jmax + 1) if jj * P > q0]
                        for i, jj in enumerate(order):
                            col0 = max(0, jj * P - q0)
                            nc.tensor.matmul(
                                po[:, col0:512],
                                lhsT=va[jj][:, h * (HS + 1):(h + 1) * (HS + 1)],
                                rhs=pts[h][jj][:, q0 + col0 - jj * P: q0 + 512 - jj * P],
                                start=(i == 0),
                                stop=(i == jmax),
                                skip_group_check=True,
                            )
                        rsum = rspool.tile([1, 512], F32, tag="rsum")
                        nc.vector.tensor_copy(rsum[:], po[HS:HS + 1, :])
                        rs = rspool.tile([1, 512], F32, tag="rs")
                        # approx is ~18 bits — plenty; needs SBUF input (the
                        # bit-twiddled seed reads raw bits, PSUM reads don't)
                        nc.vector.reciprocal_approx_fast(rs[:], rsum[:])
                        rs16 = rspool.tile([1, 512], mybir.dt.float16, tag="rs16")
                        nc.vector.tensor_copy(rs16[:], rs[:])
                        pb = pp_b.tile([HS, 512], F32, tag="pb")
                        nc.tensor.matmul(
                            pb[:],
                            lhsT=ones64[:],
                            rhs=rs16[:],
                            start=True,
                            stop=True,
                        )
                        pbs = otpool.tile([HS, 512], F32, tag="pbs")
                        nc.vector.tensor_copy(pbs[:], pb[:])
                        ot = otpool.tile([HS, 512], F32, tag="ot")
                        nc.vector.tensor_mul(ot[:], po[0:HS, :], pbs[:])
                        nc.sync.dma_start(
                            outT[h * HS:(h + 1) * HS, q0:q0 + 512], ot[:]
                        )
    nc.compile()
    return nc


_CACHE = {}


def _get_nc():
    if "nc" not in _CACHE:
        _CACHE["nc"] = _build_nc()
    return _CACHE["nc"]


def _host_inputs(x, Wq, Wk, Wv):
    bf = ml_dtypes.bfloat16
    B = x.shape[0]
    # RoPE tables (match reference: theta over hs/2 freqs with dim=n_emb)
    i = np.arange(HS // 2, dtype=np.float32)
    theta = np.float32(10000.0) ** (-2.0 * i / np.float32(CIN))
    pos = np.arange(T, dtype=np.float32)
    ang = pos[:, None] * theta[None, :]
    cosT = np.cos(ang).T.astype(np.float32)  # [32, T]
    sinT = np.sin(ang).T.astype(np.float32)
    cos4 = np.ascontiguousarray(np.tile(cosT, (4, 1)))           # [128, T]
    sin4 = np.ascontiguousarray(
        np.tile(np.concatenate([-sinT, sinT], axis=0), (2, 1))
    )  # rows: [-sin, +sin] x2
    utri_np = np.triu(np.ones((P, P), np.float32)).astype(bf)
    ident_np = np.eye(P, dtype=np.float32).astype(bf)

    perm = np.concatenate([np.arange(0, HS, 2), np.arange(1, HS, 2)])
    in_maps = []
    for core in range(8):
        b, g = core // 4, core % 4
        idx = np.concatenate([(4 * g + h) * HS + perm for h in range(NHC)])
        m = {
            "xT": np.ascontiguousarray(x[b].T).astype(bf),
            "wqT": np.ascontiguousarray(Wq[idx].T).astype(bf),
            "wkT": np.ascontiguousarray(Wk[idx].T).astype(bf),
            "wvT": np.ascontiguousarray(Wv[g * DOUT:(g + 1) * DOUT].T).astype(bf),
            "cos4": cos4,
            "sin4": sin4,
            "utri": utri_np,
            "ident": ident_np,
        }
        in_maps.append(m)
    return in_maps


def kernel(x, Wq, Wk, Wv, _trace=False, _trace_kwargs=None):
    x = np.asarray(x)
    Wq, Wk, Wv = np.asarray(Wq), np.asarray(Wk), np.asarray(Wv)
    B = x.shape[0]
    nc = _get_nc()
    in_maps = _host_inputs(x, Wq, Wk, Wv)
    res = run_bass_kernel_spmd(
        nc, in_maps, list(range(8)), trace=_trace, **(_trace_kwargs or {})
    )
    out = np.zeros((B, T, CIN), np.float32)
    for core in range(8):
        b, g = core // 4, core % 4
        out[b, :, g * DOUT:(g + 1) * DOUT] = res.results[core]["outT"].T
    if _trace:
        return out, res
    return out

